# revision 31
# baseline (speedup 1.0000x reference)
"""Trainium2 Bass kernel for CustomLSTMModel.

Model: tokens [256,512] -> embedding (padding_idx=0) -> 1-layer LSTM(300->512)
       -> last hidden state -> FC(512->7).

Strategy (8 NeuronCores, data-parallel over batch, 32 rows/core):
  - Gates PSUM tile [128, 512]: partitions = (hidden-sub-block i, batch b),
    free = (gate block [f|i|o|g], quarter q, hidden-mod-32 a). Each of the 4
    PE column-group matmuls (tile_position=(0,32i)) uses its OWN host-side
    weight-column permutation so partition block i holds hidden dims
    128q+32i+a. With this layout the next-step lhsT (h^T, K-major) is
    obtained with ONE DVE stream-transpose (32x32 blocks, in-place) right
    after the h-mult on the same engine - no PE transpose, no PSUM
    round-trip, no extra copies or semaphore hops.
  - h-rounds (bf16, 4 K-rounds x 4 col-groups); only the last K-round is
    split [f,i,g | o]. PE completion-semaphore increments drain at ~41ns
    each, so the sigmoid's wait is drain-bound at the 16th h-matmul -
    instruction count on this phase is deliberately minimal.
  - x-projection: 3 bf16 K-rounds (128/128/45+bias) per step, issued a step
    ahead so they execute during the elementwise phase.
  - Elementwise all-bf16 (c kept bf16): g columns pre-scaled x2 in the
    weights so ONE sigmoid covers [f,i,g] (tanh(g) = 2*sig(2g)-1, fixed up
    inside the fused scalar_tensor_tensor cell update); tanh(c)/h-mult/
    transpose run in halves so k-rounds 0,1 launch while the second half
    finishes; sig(o) runs in the ACT gap.
  - FILLER dummy matmuls keep the PE continuously busy through the
    elementwise phase so it holds the 2.4 GHz p-state (PE drops to 1.2 GHz
    within ~100ns of idling and needs 3us of continuous work to re-ramp).
  - Embedding rows gathered 128 at a time by indirect DMA, PE-transposed to
    K-major (4 steps per gather group).
"""
import numpy as np
import ml_dtypes

import concourse.bass as bass
import concourse.tile as tile
from concourse import bacc, mybir
from concourse.bass_utils import run_bass_kernel_spmd

BF16 = mybir.dt.bfloat16
F32 = mybir.dt.float32
I32 = mybir.dt.int32

B, S, E, H, OUT = 256, 512, 300, 512, 7
NCORES = 8
BC = B // NCORES          # batch per core (32)
NG = S // 4               # token groups of 128 = 4 steps
PREFETCH = 2              # gather groups in flight ahead
FILLER = 16                # dummy PE rounds per step to hold max p-state

_BUILD_CACHE = {}


def _build(n_steps=S, filler=FILLER):
    key = (n_steps, filler)
    if key in _BUILD_CACHE:
        return _BUILD_CACHE[key]
    ngroups = (n_steps + 3) // 4
    nc = bacc.Bacc("TRN2", target_bir_lowering=False, debug=False)

    wx = nc.dram_tensor("wx", [3, 128, 2048], BF16, kind="ExternalInput")
    whh = nc.dram_tensor("whh", [4, 128, 2048], BF16, kind="ExternalInput")
    emb = nc.dram_tensor("emb", [32000, E], BF16, kind="ExternalInput")
    toks = nc.dram_tensor("toks", [NG, 128, 1], I32, kind="ExternalInput")
    identx = nc.dram_tensor("identx", [128, 128], BF16, kind="ExternalInput")
    wfct = nc.dram_tensor("wfct", [4, 128, OUT], BF16, kind="ExternalInput")
    bfc = nc.dram_tensor("bfc", [1, OUT], BF16, kind="ExternalInput")
    logits = nc.dram_tensor("logits", [BC, OUT], F32, kind="ExternalOutput")

    SIG = mybir.ActivationFunctionType.Sigmoid
    TANH = mybir.ActivationFunctionType.Tanh

    with tile.TileContext(nc) as tc:
        with (
            tc.tile_pool(name="const", bufs=1) as cpool,
            tc.tile_pool(name="xg", bufs=PREFETCH + 2) as xpool,
            tc.tile_pool(name="work", bufs=2) as wpool,
            tc.tile_pool(name="hT", bufs=4) as hpool,
            tc.tile_pool(name="gpsum", bufs=3, space="PSUM") as gpsum,
            tc.tile_pool(name="fpsum", bufs=1, space="PSUM") as fpsum,
            tc.tile_pool(name="xpsum", bufs=3, space="PSUM") as xpsum,
        ):
            # ---- constants ----
            wx_sb = []
            for r in range(3):
                wt = cpool.tile([128, 2048], BF16, tag=f"wx{r}")
                nc.sync.dma_start(wt[:], wx.ap()[r])
                wx_sb.append(wt)
            whh_sb = []
            for k in range(4):
                wt = cpool.tile([128, 2048], BF16, tag=f"whh{k}")
                nc.sync.dma_start(wt[:], whh.ap()[k])
                whh_sb.append(wt)
            identx_sb = cpool.tile([128, 128], BF16, tag="identx")
            nc.sync.dma_start(identx_sb[:], identx.ap())
            wfct_sb = []
            for k in range(4):
                wf = cpool.tile([128, OUT], BF16, tag=f"wfct{k}")
                nc.sync.dma_start(wf[:], wfct.ap()[k])
                wfct_sb.append(wf)
            bfc_sb = cpool.tile([1, OUT], BF16, tag="bfc")
            nc.sync.dma_start(bfc_sb[:], bfc.ap())
            ones_sb = cpool.tile([1, 32], BF16, tag="ones")
            nc.gpsimd.memset(ones_sb[:], 1.0)

            # persistent cell state: c in cols 0:128, tanh(g) lands in 128:256
            ct = cpool.tile([128, 256], BF16, tag="ct")
            nc.gpsimd.memset(ct[:, 0:128], 0.0)

            fill_ps = fpsum.tile([128, 256], F32, tag="fill")

            # ---- x pipeline: gather 128 emb rows -> transpose to K-major ----
            def prefetch(g):
                tok_sb = xpool.tile([128, 1], I32, tag="tok")
                nc.sync.dma_start(tok_sb[:], toks.ap()[g])
                x_sb = xpool.tile([128, 304], BF16, tag="xsb")
                nc.gpsimd.memset(x_sb[:, 300:301], 1.0)
                nc.gpsimd.indirect_dma_start(
                    out=x_sb[:, 0:E],
                    out_offset=None,
                    in_=emb.ap(),
                    in_offset=bass.IndirectOffsetOnAxis(ap=tok_sb[:, :1], axis=0),
                )
                xp = xpsum.tile([128, 384], BF16, tag="xp")
                for s_i in range(3):
                    w = min(128, 301 - 128 * s_i)  # 128,128,45 (45th = ones)
                    nc.tensor.transpose(
                        out=xp[0:w, 128 * s_i:128 * s_i + 128],
                        in_=x_sb[:, 128 * s_i:128 * s_i + w],
                        identity=identx_sb[:],
                        tile_position=(0, 0),
                    )
                xg = xpool.tile([128, 384], BF16, tag="xgall")
                nc.vector.tensor_copy(xg[:, 0:256], xp[:, 0:256])
                nc.vector.tensor_copy(xg[0:45, 256:384], xp[0:45, 256:384])
                return xg

            xg_tiles = {}
            for g in range(min(PREFETCH, ngroups)):
                xg_tiles[g] = prefetch(g)

            def emit_xr(t, first):
                """x-projection rounds of step t into a fresh gates tile."""
                g, lt = t // 4, t % 4
                if lt == 0 and g + PREFETCH < ngroups:
                    xg_tiles[g + PREFETCH] = prefetch(g + PREFETCH)
                xg = xg_tiles[g]
                gates = gpsum.tile([128, 512], F32, tag="gates")
                rounds = [
                    (xg[0:128, 0 + 32 * lt:0 + 32 * lt + 32], wx_sb[0][:]),
                    (xg[0:128, 128 + 32 * lt:128 + 32 * lt + 32], wx_sb[1][:]),
                    (xg[0:45, 256 + 32 * lt:256 + 32 * lt + 32], wx_sb[2][0:45, :]),
                ]
                for r, (lh, wt) in enumerate(rounds):
                    for j in range(4):
                        nc.tensor.matmul(
                            out=gates[32 * j:32 * (j + 1), :], lhsT=lh,
                            rhs=wt[:, 512 * j:512 * (j + 1)],
                            start=(r == 0), stop=(first and r == 2),
                            tile_position=(0, 32 * j), skip_group_check=True)
                return gates

            hT = None
            gates_q = {0: emit_xr(0, first=True)}
            for t in range(n_steps):
                gates = gates_q.pop(t)
                # ---- recurrent rounds: 4 K-rounds x 4 column-group matmuls.
                # 20 instructions total: PE completions drain at ~41ns each
                # into the semaphore the sigmoid waits on, so instruction
                # count on this phase is precious. Only the LAST K-round is
                # split [f,i,g | o] (the sigmoid needs just cols 0:384, so
                # its wait lands on the 16th instruction, ~56ns earlier). ----
                if hT is not None:
                    for k in range(3):
                        for j in range(4):
                            nc.tensor.matmul(
                                out=gates[32 * j:32 * (j + 1), :],
                                lhsT=hT[:, 32 * k:32 * k + 32],
                                rhs=whh_sb[k][:, 512 * j:512 * (j + 1)],
                                start=False, stop=False,
                                tile_position=(0, 32 * j),
                                skip_group_check=True)
                    for lo, w in ((0, 384), (384, 128)):
                        for j in range(4):
                            nc.tensor.matmul(
                                out=gates[32 * j:32 * (j + 1), lo:lo + w],
                                lhsT=hT[:, 96:128],
                                rhs=whh_sb[3][:, 512 * j + lo:512 * j + lo + w],
                                start=False, stop=True,
                                tile_position=(0, 32 * j),
                                skip_group_check=True)

                # ---- elementwise (bf16): cols [f | i | g2 | o] blocks.
                # g columns are pre-scaled x2 in the weights so one sigmoid
                # covers f,i,g: tanh(g) = 2*sig(2g) - 1 (DVE fixes up). ----
                sg = wpool.tile([128, 512], BF16, tag="sig")
                nc.scalar.activation(out=sg[:, 0:384], in_=gates[:, 0:384],
                                     func=SIG)
                nc.scalar.activation(out=sg[:, 384:512], in_=gates[:, 384:512],
                                     func=SIG)
                # cell update via fused scalar_tensor_tensor ops:
                #   fc  = sig(f) * c
                #   pih = (sig(2g) - 0.5) * sig(i)      [= i*tanh(g)/2]
                #   c   = 2*pih + fc
                tmp = wpool.tile([128, 256], BF16, tag="tmp")
                nc.vector.tensor_tensor(out=tmp[:, 0:128], in0=sg[:, 0:128],
                                        in1=ct[:, 0:128],
                                        op=mybir.AluOpType.mult)
                nc.vector.scalar_tensor_tensor(
                    out=tmp[:, 128:256], in0=sg[:, 256:384], scalar=0.5,
                    in1=sg[:, 128:256], op0=mybir.AluOpType.subtract,
                    op1=mybir.AluOpType.mult)
                for lo in (0, 64):
                    nc.vector.scalar_tensor_tensor(
                        out=ct[:, lo:lo + 64], in0=tmp[:, 128 + lo:192 + lo],
                        scalar=2.0, in1=tmp[:, lo:lo + 64],
                        op0=mybir.AluOpType.mult, op1=mybir.AluOpType.add)
                # tanh(c) -> h -> transpose in halves: k-rounds 0,1 need only
                # hT cols 0:64, so they launch while the second half finishes
                tc_t = wpool.tile([128, 128], BF16, tag="tanhc")
                h_bf = wpool.tile([128, 128], BF16, tag="hwide")
                hT = hpool.tile([128, 128], BF16, tag="hT")
                for lo in (0, 64):
                    nc.scalar.activation(out=tc_t[:, lo:lo + 64],
                                         in_=ct[:, lo:lo + 64], func=TANH)
                for lo in (0, 64):
                    nc.vector.tensor_tensor(out=h_bf[:, lo:lo + 64],
                                            in0=sg[:, 384 + lo:448 + lo],
                                            in1=tc_t[:, lo:lo + 64],
                                            op=mybir.AluOpType.mult)
                    nc.vector.transpose(out=hT[:, lo:lo + 64],
                                        in_=h_bf[:, lo:lo + 64])

                # next step's x rounds + fillers keep the PE busy (p-state)
                if t + 1 < n_steps:
                    gates_q[t + 1] = emit_xr(t + 1, first=False)
                # fillers rotate across column groups so no group's queue
                # delays the next real matmul in that group
                for fi in range(filler):
                    fj = fi % 4
                    nc.tensor.matmul(
                        out=fill_ps[32 * fj:32 * fj + 32, 0:256],
                        lhsT=identx_sb[:, 0:32],
                        rhs=wx_sb[0][:, 0:256], start=True, stop=True,
                        tile_position=(0, 32 * fj), skip_group_check=True)

            # ---- FC head: logits = h_T @ W_fc.T + b_fc ----
            fc_ps = gpsum.tile([32, OUT], F32, tag="gates")
            for k in range(4):
                nc.tensor.matmul(out=fc_ps[:], lhsT=hT[:, 32 * k:32 * k + 32],
                                 rhs=wfct_sb[k][:], start=(k == 0), stop=False,
                                 tile_position=(0, 0))
            nc.tensor.matmul(out=fc_ps[:], lhsT=ones_sb[:], rhs=bfc_sb[:],
                             start=False, stop=True, tile_position=(0, 0))
            fc_sb = wpool.tile([32, OUT], F32, tag="fcout")
            nc.scalar.copy(out=fc_sb[:], in_=fc_ps[:])
            nc.sync.dma_start(logits.ap(), fc_sb[:])

    nc.compile()
    _BUILD_CACHE[key] = nc
    return nc


def _prep_inputs(tokens, emb, W_ih, b_ih, W_hh, b_hh, W_fc, b_fc, n_steps=S):
    """Host-side weight packing (dtype casts, transposes, gate permutation)."""
    bf = ml_dtypes.bfloat16
    # per-quadrant gate column permutation: our col 512*i + 128*g + 32*q + a
    # (i = hidden-sub-block / PE col-group, g = gate [f,i,g,o], q = quarter,
    # a = hidden mod 32) maps to orig col 512*go + 128*q + 32*i + a
    perm = np.empty(2048, np.int64)
    go_of_g = [1, 0, 2, 3]   # [f, i, g, o] -> PyTorch [i, f, g, o] rows
    ar = np.arange(32)
    for i in range(4):
        for g in range(4):
            for q in range(4):
                base = 512 * i + 128 * g + 32 * q
                perm[base:base + 32] = 512 * go_of_g[g] + 128 * q + 32 * i + ar
    # g block pre-scaled x2: tanh(g) computed as 2*sigmoid(2g) - 1
    gscale = np.ones(2048, np.float32)
    for i in range(4):
        gscale[512 * i + 256:512 * i + 384] = 2.0

    WihT = W_ih.T.astype(np.float32)[:, perm] * gscale  # [300, 2048]
    WhhT = W_hh.T.astype(np.float32)[:, perm] * gscale  # [512, 2048]
    bias = (b_ih + b_hh).astype(np.float32)[perm] * gscale  # [2048]

    wx = np.zeros((3, 128, 2048), np.float32)
    wx[0] = WihT[0:128]
    wx[1] = WihT[128:256]
    wx[2][0:44] = WihT[256:300]
    wx[2][44] = bias
    wx = wx.astype(bf)

    whh = np.ascontiguousarray(WhhT.reshape(4, 128, 2048)).astype(bf)

    emb0 = emb.astype(np.float32).copy()
    emb0[0] = 0.0
    emb_bf = emb0.astype(bf)

    identx = np.eye(128, dtype=bf)
    # FC head consumes hT whose partition layout is (sub-block i, batch b)
    # only via 32-col slices k: lhsT_k[p, b] = h[b, 128k+p] -> W_fc cols must
    # be in plain hidden order chunked by k
    wfct = np.ascontiguousarray(
        W_fc.T.astype(np.float32).reshape(4, 128, OUT)).astype(bf)
    bfc = b_fc.astype(np.float32).reshape(1, OUT).astype(bf)

    in_maps = []
    for core in range(NCORES):
        tcore = tokens[core * BC:(core + 1) * BC]          # [32, 512]
        tg = np.ascontiguousarray(tcore.T)                 # [512, 32] (t, b)
        tg = tg.reshape(NG, 4 * BC, 1).astype(np.int32)    # [(g), (lt,b), 1]
        in_maps.append({
            "wx": wx, "whh": whh, "emb": emb_bf, "toks": tg,
            "identx": identx, "wfct": wfct, "bfc": bfc,
        })
    return in_maps


def kernel(tokens, emb, W_ih, b_ih, W_hh, b_hh, W_fc, b_fc, n_steps=S,
           profile=False):
    nc = _build(n_steps)
    in_maps = _prep_inputs(tokens, emb, W_ih, b_ih, W_hh, b_hh, W_fc, b_fc,
                           n_steps=n_steps)
    kw = {}
    if profile:
        kw = dict(trace=True, tmpdir="/tmp/lstm_trace")
    res = run_bass_kernel_spmd(nc, in_maps, list(range(NCORES)), **kw)
    out = np.concatenate([res.results[i]["logits"] for i in range(NCORES)], axis=0)
    if profile:
        kernel.last_exec_time_ns = res.exec_time_ns
        kernel.last_results = res
    return out.astype(np.float32)


# revision 32
# speedup vs baseline: 1.0026x; 1.0026x over previous
"""Trainium2 Bass kernel for CustomLSTMModel.

Model: tokens [256,512] -> embedding (padding_idx=0) -> 1-layer LSTM(300->512)
       -> last hidden state -> FC(512->7).

Strategy (8 NeuronCores, data-parallel over batch, 32 rows/core):
  - Gates PSUM tile [128, 512]: partitions = (hidden-sub-block i, batch b),
    free = (gate block [f|i|o|g], quarter q, hidden-mod-32 a). Each of the 4
    PE column-group matmuls (tile_position=(0,32i)) uses its OWN host-side
    weight-column permutation so partition block i holds hidden dims
    128q+32i+a. With this layout the next-step lhsT (h^T, K-major) is
    obtained with ONE DVE stream-transpose (32x32 blocks, in-place) right
    after the h-mult on the same engine - no PE transpose, no PSUM
    round-trip, no extra copies or semaphore hops.
  - h-rounds (bf16, 4 K-rounds x 4 col-groups); only the last K-round is
    split [f,i,g | o]. PE completion-semaphore increments drain at ~41ns
    each, so the sigmoid's wait is drain-bound at the 16th h-matmul -
    instruction count on this phase is deliberately minimal.
  - x-projection: 3 bf16 K-rounds (128/128/45+bias) per step, issued a step
    ahead so they execute during the elementwise phase.
  - Elementwise all-bf16 (c kept bf16): g columns pre-scaled x2 in the
    weights so ONE sigmoid covers [f,i,g] (tanh(g) = 2*sig(2g)-1, fixed up
    inside the fused scalar_tensor_tensor cell update); tanh(c)/h-mult/
    transpose run in halves so k-rounds 0,1 launch while the second half
    finishes; sig(o) runs in the ACT gap.
  - FILLER dummy matmuls keep the PE continuously busy through the
    elementwise phase so it holds the 2.4 GHz p-state (PE drops to 1.2 GHz
    within ~100ns of idling and needs 3us of continuous work to re-ramp).
  - Embedding rows gathered 128 at a time by indirect DMA, PE-transposed to
    K-major (4 steps per gather group).
"""
import numpy as np
import ml_dtypes

import concourse.bass as bass
import concourse.tile as tile
from concourse import bacc, mybir
from concourse.bass_utils import run_bass_kernel_spmd

BF16 = mybir.dt.bfloat16
F32 = mybir.dt.float32
I32 = mybir.dt.int32

B, S, E, H, OUT = 256, 512, 300, 512, 7
NCORES = 8
BC = B // NCORES          # batch per core (32)
NG = S // 4               # token groups of 128 = 4 steps
PREFETCH = 2              # gather groups in flight ahead
FILLER = 16                # dummy PE rounds per step to hold max p-state

_BUILD_CACHE = {}


def _build(n_steps=S, filler=FILLER):
    key = (n_steps, filler)
    if key in _BUILD_CACHE:
        return _BUILD_CACHE[key]
    ngroups = (n_steps + 3) // 4
    nc = bacc.Bacc("TRN2", target_bir_lowering=False, debug=False)

    wx = nc.dram_tensor("wx", [3, 128, 2048], BF16, kind="ExternalInput")
    whh = nc.dram_tensor("whh", [4, 128, 2048], BF16, kind="ExternalInput")
    emb = nc.dram_tensor("emb", [32000, E], BF16, kind="ExternalInput")
    toks = nc.dram_tensor("toks", [NG, 128, 1], I32, kind="ExternalInput")
    identx = nc.dram_tensor("identx", [128, 128], BF16, kind="ExternalInput")
    wfct = nc.dram_tensor("wfct", [4, 128, OUT], BF16, kind="ExternalInput")
    bfc = nc.dram_tensor("bfc", [1, OUT], BF16, kind="ExternalInput")
    logits = nc.dram_tensor("logits", [BC, OUT], F32, kind="ExternalOutput")

    SIG = mybir.ActivationFunctionType.Sigmoid
    TANH = mybir.ActivationFunctionType.Tanh

    with tile.TileContext(nc) as tc:
        with (
            tc.tile_pool(name="const", bufs=1) as cpool,
            tc.tile_pool(name="xg", bufs=PREFETCH + 2) as xpool,
            tc.tile_pool(name="work", bufs=2) as wpool,
            tc.tile_pool(name="hT", bufs=4) as hpool,
            tc.tile_pool(name="gpsum", bufs=3, space="PSUM") as gpsum,
            tc.tile_pool(name="fpsum", bufs=1, space="PSUM") as fpsum,
            tc.tile_pool(name="xpsum", bufs=2, space="PSUM") as xpsum,
        ):
            # ---- constants ----
            wx_sb = []
            for r in range(3):
                wt = cpool.tile([128, 2048], BF16, tag=f"wx{r}")
                nc.sync.dma_start(wt[:], wx.ap()[r])
                wx_sb.append(wt)
            whh_sb = []
            for k in range(4):
                wt = cpool.tile([128, 2048], BF16, tag=f"whh{k}")
                nc.sync.dma_start(wt[:], whh.ap()[k])
                whh_sb.append(wt)
            identx_sb = cpool.tile([128, 128], BF16, tag="identx")
            nc.sync.dma_start(identx_sb[:], identx.ap())
            wfct_sb = []
            for k in range(4):
                wf = cpool.tile([128, OUT], BF16, tag=f"wfct{k}")
                nc.sync.dma_start(wf[:], wfct.ap()[k])
                wfct_sb.append(wf)
            bfc_sb = cpool.tile([1, OUT], BF16, tag="bfc")
            nc.sync.dma_start(bfc_sb[:], bfc.ap())
            ones_sb = cpool.tile([1, 32], BF16, tag="ones")
            nc.gpsimd.memset(ones_sb[:], 1.0)

            # persistent cell state: c in cols 0:128, tanh(g) lands in 128:256
            ct = cpool.tile([128, 256], BF16, tag="ct")
            nc.gpsimd.memset(ct[:, 0:128], 0.0)

            fill_ps = fpsum.tile([128, 256], F32, tag="fill")

            # ---- x pipeline: gather 128 emb rows -> transpose to K-major ----
            def prefetch(g):
                tok_sb = xpool.tile([128, 1], I32, tag="tok")
                nc.sync.dma_start(tok_sb[:], toks.ap()[g])
                x_sb = xpool.tile([128, 304], BF16, tag="xsb")
                nc.gpsimd.memset(x_sb[:, 300:301], 1.0)
                nc.gpsimd.indirect_dma_start(
                    out=x_sb[:, 0:E],
                    out_offset=None,
                    in_=emb.ap(),
                    in_offset=bass.IndirectOffsetOnAxis(ap=tok_sb[:, :1], axis=0),
                )
                xp = xpsum.tile([128, 384], BF16, tag="xp")
                for s_i in range(3):
                    w = min(128, 301 - 128 * s_i)  # 128,128,45 (45th = ones)
                    nc.tensor.transpose(
                        out=xp[0:w, 128 * s_i:128 * s_i + 128],
                        in_=x_sb[:, 128 * s_i:128 * s_i + w],
                        identity=identx_sb[:],
                        tile_position=(0, 0),
                    )
                xg = xpool.tile([128, 384], BF16, tag="xgall")
                nc.vector.tensor_copy(xg[:, 0:256], xp[:, 0:256])
                nc.vector.tensor_copy(xg[0:45, 256:384], xp[0:45, 256:384])
                return xg

            xg_tiles = {}
            for g in range(min(PREFETCH, ngroups)):
                xg_tiles[g] = prefetch(g)

            def emit_xr(t, first):
                """x-projection rounds of step t into a fresh gates tile."""
                g, lt = t // 4, t % 4
                if lt == 0 and g + PREFETCH < ngroups:
                    xg_tiles[g + PREFETCH] = prefetch(g + PREFETCH)
                xg = xg_tiles[g]
                gates = gpsum.tile([128, 512], F32, tag="gates")
                rounds = [
                    (xg[0:128, 0 + 32 * lt:0 + 32 * lt + 32], wx_sb[0][:]),
                    (xg[0:128, 128 + 32 * lt:128 + 32 * lt + 32], wx_sb[1][:]),
                    (xg[0:45, 256 + 32 * lt:256 + 32 * lt + 32], wx_sb[2][0:45, :]),
                ]
                for r, (lh, wt) in enumerate(rounds):
                    for j in range(4):
                        nc.tensor.matmul(
                            out=gates[32 * j:32 * (j + 1), :], lhsT=lh,
                            rhs=wt[:, 512 * j:512 * (j + 1)],
                            start=(r == 0), stop=(first and r == 2),
                            tile_position=(0, 32 * j), skip_group_check=True)
                return gates

            hT = None
            gates_q = {0: emit_xr(0, first=True)}
            for t in range(n_steps):
                gates = gates_q.pop(t)
                # ---- recurrent rounds: 4 K-rounds x 4 column-group matmuls.
                # 20 instructions total: PE completions drain at ~41ns each
                # into the semaphore the sigmoid waits on, so instruction
                # count on this phase is precious. Only the LAST K-round is
                # split [f,i,g | o] (the sigmoid needs just cols 0:384, so
                # its wait lands on the 16th instruction, ~56ns earlier). ----
                if hT is not None:
                    for k in range(3):
                        for j in range(4):
                            nc.tensor.matmul(
                                out=gates[32 * j:32 * (j + 1), :],
                                lhsT=hT[:, 32 * k:32 * k + 32],
                                rhs=whh_sb[k][:, 512 * j:512 * (j + 1)],
                                start=False, stop=False,
                                tile_position=(0, 32 * j),
                                skip_group_check=True)
                    for lo, w in ((0, 384), (384, 128)):
                        for j in range(4):
                            nc.tensor.matmul(
                                out=gates[32 * j:32 * (j + 1), lo:lo + w],
                                lhsT=hT[:, 96:128],
                                rhs=whh_sb[3][:, 512 * j + lo:512 * j + lo + w],
                                start=False, stop=True,
                                tile_position=(0, 32 * j),
                                skip_group_check=True)

                # ---- elementwise (bf16): cols [f | i | g2 | o] blocks.
                # g columns are pre-scaled x2 in the weights so one sigmoid
                # covers f,i,g: tanh(g) = 2*sig(2g) - 1 (DVE fixes up). ----
                sg = wpool.tile([128, 512], BF16, tag="sig")
                nc.scalar.activation(out=sg[:, 0:384], in_=gates[:, 0:384],
                                     func=SIG)
                nc.scalar.activation(out=sg[:, 384:512], in_=gates[:, 384:512],
                                     func=SIG)
                # cell update via fused scalar_tensor_tensor ops:
                #   fc  = sig(f) * c
                #   pih = (sig(2g) - 0.5) * sig(i)      [= i*tanh(g)/2]
                #   c   = 2*pih + fc
                tmp = wpool.tile([128, 256], BF16, tag="tmp")
                nc.vector.tensor_tensor(out=tmp[:, 0:128], in0=sg[:, 0:128],
                                        in1=ct[:, 0:128],
                                        op=mybir.AluOpType.mult)
                nc.vector.scalar_tensor_tensor(
                    out=tmp[:, 128:256], in0=sg[:, 256:384], scalar=0.5,
                    in1=sg[:, 128:256], op0=mybir.AluOpType.subtract,
                    op1=mybir.AluOpType.mult)
                for lo in (0, 64):
                    nc.vector.scalar_tensor_tensor(
                        out=ct[:, lo:lo + 64], in0=tmp[:, 128 + lo:192 + lo],
                        scalar=2.0, in1=tmp[:, lo:lo + 64],
                        op0=mybir.AluOpType.mult, op1=mybir.AluOpType.add)
                # tanh(c) -> h -> transpose in halves: k-rounds 0,1 need only
                # hT cols 0:64, so they launch while the second half finishes
                tc_t = wpool.tile([128, 128], BF16, tag="tanhc")
                h_bf = wpool.tile([128, 128], BF16, tag="hwide")
                hT = hpool.tile([128, 128], BF16, tag="hT")
                for lo in (0, 64):
                    nc.scalar.activation(out=tc_t[:, lo:lo + 64],
                                         in_=ct[:, lo:lo + 64], func=TANH)
                for lo in (0, 64):
                    nc.vector.tensor_tensor(out=h_bf[:, lo:lo + 64],
                                            in0=sg[:, 384 + lo:448 + lo],
                                            in1=tc_t[:, lo:lo + 64],
                                            op=mybir.AluOpType.mult)
                    nc.vector.transpose(out=hT[:, lo:lo + 64],
                                        in_=h_bf[:, lo:lo + 64])

                # next step's x rounds + fillers keep the PE busy (p-state)
                if t + 1 < n_steps:
                    gates_q[t + 1] = emit_xr(t + 1, first=False)
                # fillers rotate across column groups so no group's queue
                # delays the next real matmul in that group
                for fi in range(filler):
                    fj = fi % 4
                    nc.tensor.matmul(
                        out=fill_ps[32 * fj:32 * fj + 32, 0:256],
                        lhsT=identx_sb[:, 0:32],
                        rhs=wx_sb[0][:, 0:256], start=True, stop=True,
                        tile_position=(0, 32 * fj), skip_group_check=True)

            # ---- FC head: logits = h_T @ W_fc.T + b_fc ----
            fc_ps = gpsum.tile([32, OUT], F32, tag="gates")
            for k in range(4):
                nc.tensor.matmul(out=fc_ps[:], lhsT=hT[:, 32 * k:32 * k + 32],
                                 rhs=wfct_sb[k][:], start=(k == 0), stop=False,
                                 tile_position=(0, 0))
            nc.tensor.matmul(out=fc_ps[:], lhsT=ones_sb[:], rhs=bfc_sb[:],
                             start=False, stop=True, tile_position=(0, 0))
            fc_sb = wpool.tile([32, OUT], F32, tag="fcout")
            nc.scalar.copy(out=fc_sb[:], in_=fc_ps[:])
            nc.sync.dma_start(logits.ap(), fc_sb[:])

    nc.compile()
    _BUILD_CACHE[key] = nc
    return nc


def _prep_inputs(tokens, emb, W_ih, b_ih, W_hh, b_hh, W_fc, b_fc, n_steps=S):
    """Host-side weight packing (dtype casts, transposes, gate permutation)."""
    bf = ml_dtypes.bfloat16
    # per-quadrant gate column permutation: our col 512*i + 128*g + 32*q + a
    # (i = hidden-sub-block / PE col-group, g = gate [f,i,g,o], q = quarter,
    # a = hidden mod 32) maps to orig col 512*go + 128*q + 32*i + a
    perm = np.empty(2048, np.int64)
    go_of_g = [1, 0, 2, 3]   # [f, i, g, o] -> PyTorch [i, f, g, o] rows
    ar = np.arange(32)
    for i in range(4):
        for g in range(4):
            for q in range(4):
                base = 512 * i + 128 * g + 32 * q
                perm[base:base + 32] = 512 * go_of_g[g] + 128 * q + 32 * i + ar
    # g block pre-scaled x2: tanh(g) computed as 2*sigmoid(2g) - 1
    gscale = np.ones(2048, np.float32)
    for i in range(4):
        gscale[512 * i + 256:512 * i + 384] = 2.0

    WihT = W_ih.T.astype(np.float32)[:, perm] * gscale  # [300, 2048]
    WhhT = W_hh.T.astype(np.float32)[:, perm] * gscale  # [512, 2048]
    bias = (b_ih + b_hh).astype(np.float32)[perm] * gscale  # [2048]

    wx = np.zeros((3, 128, 2048), np.float32)
    wx[0] = WihT[0:128]
    wx[1] = WihT[128:256]
    wx[2][0:44] = WihT[256:300]
    wx[2][44] = bias
    wx = wx.astype(bf)

    whh = np.ascontiguousarray(WhhT.reshape(4, 128, 2048)).astype(bf)

    emb0 = emb.astype(np.float32).copy()
    emb0[0] = 0.0
    emb_bf = emb0.astype(bf)

    identx = np.eye(128, dtype=bf)
    # FC head consumes hT whose partition layout is (sub-block i, batch b)
    # only via 32-col slices k: lhsT_k[p, b] = h[b, 128k+p] -> W_fc cols must
    # be in plain hidden order chunked by k
    wfct = np.ascontiguousarray(
        W_fc.T.astype(np.float32).reshape(4, 128, OUT)).astype(bf)
    bfc = b_fc.astype(np.float32).reshape(1, OUT).astype(bf)

    in_maps = []
    for core in range(NCORES):
        tcore = tokens[core * BC:(core + 1) * BC]          # [32, 512]
        tg = np.ascontiguousarray(tcore.T)                 # [512, 32] (t, b)
        tg = tg.reshape(NG, 4 * BC, 1).astype(np.int32)    # [(g), (lt,b), 1]
        in_maps.append({
            "wx": wx, "whh": whh, "emb": emb_bf, "toks": tg,
            "identx": identx, "wfct": wfct, "bfc": bfc,
        })
    return in_maps


def kernel(tokens, emb, W_ih, b_ih, W_hh, b_hh, W_fc, b_fc, n_steps=S,
           profile=False):
    nc = _build(n_steps)
    in_maps = _prep_inputs(tokens, emb, W_ih, b_ih, W_hh, b_hh, W_fc, b_fc,
                           n_steps=n_steps)
    kw = {}
    if profile:
        kw = dict(trace=True, tmpdir="/tmp/lstm_trace")
    res = run_bass_kernel_spmd(nc, in_maps, list(range(NCORES)), **kw)
    out = np.concatenate([res.results[i]["logits"] for i in range(NCORES)], axis=0)
    if profile:
        kernel.last_exec_time_ns = res.exec_time_ns
        kernel.last_results = res
    return out.astype(np.float32)
